# revision 2
# baseline (speedup 1.0000x reference)
"""Trainium2 Bass kernel for a 2-layer LSTM (B=256, T=512, I=64, H=256) + linear head.

Strategy (hardcoded, self-contained):
  - Data-parallel over batch across 8 NeuronCores (32 batch elems per core).
  - Per core, both LSTM layers run step-by-step in a feature-blocked layout:
      gate PSUM tile [128=(hblk4, b32), 256=(gate4, hh2, hl32)]
    produced by col-group-packed fp16 matmuls (tile_position=(0, 32*m)) that
    share the small transposed-state stationary hT [k, 32].
  - All matmul operands are fp16 (PSUM accumulation stays fp32): 4x faster
    streaming than fp32 on the PE.
  - The gate nonlinearity is a SINGLE sigmoid per gate tile: tanh is folded
    into sigmoid via tanh(x) = 2*sigmoid(2x) - 1, with the 2x pre-scale folded
    into the g-gate weight columns, the cell state kept as C = 2c, and the
    output scale folded into downstream weights via h' = h/2 (consumer
    weights x2).
  - Elementwise runs as fused scalar_tensor_tensor DVE ops in fp16 (4x DVE
    mode: 2-byte dtype, all-SBUF operands).
  - Input projection x@Wih.T and biases ride the same PSUM accumulation
    (augmented ones-row trick).
  - The two output linear layers are folded host-side into a single
    [256, 4] matmul + bias.
  - All weights ship as ONE packed fp16 DRAM blob -> one DMA.
"""

import numpy as np

B, T, I, H, O = 256, 512, 64, 256, 4
NCORES = 8
BS = B // NCORES  # 32

# reference gate order is (i, f, g, o); we reorder to (i, f, o, g) so that
# cols 0:64=i, 64:128=f, 128:192=o, 192:256=g per m-block.
GATE_PERM = [0, 1, 3, 2]

# weight blob column offsets (fp16 elements, [128, WB_COLS])
OFF_W0 = 0        # Whh0 perm  [128, 2*1024]
OFF_W1 = 2048     # Whh1 perm  [128, 2*1024]
OFF_WX1 = 4096    # Wih1 perm  [128, 2*1024]
OFF_WX0 = 6144    # Wih0 perm + bias row, rows 0:65, [65, 1024]
OFF_B1 = 7168     # bias1 row, row 0, [1, 1024]
OFF_WF = 8192     # folded head weight [128, 2*4]
OFF_BF = 8200     # folded head bias, row 0, [1, 4]
OFF_XT = 8224     # x transposed + ones row, rows 0:65, [65, t_steps*32]
def _wb_cols(t_steps):
    return OFF_XT + t_steps * BS

_CACHED = {}


def _perm_cols(Wt):
    """Permute gate columns of [K, 1024] (col j = gate_orig*256 + h) into
    col = m*256 + gate_new*64 + hh*32 + hl, where h = hh*128 + m*32 + hl."""
    K = Wt.shape[0]
    W = Wt.reshape(K, 4, 256)[:, GATE_PERM, :]      # [K, gate, h]
    W = W.reshape(K, 4, 2, 4, 32)                    # [K, gate, hh, m, hl]
    W = W.transpose(0, 3, 1, 2, 4)                   # [K, m, gate, hh, hl]
    return np.ascontiguousarray(W.reshape(K, 1024))


def _build_bass(t_steps=T):
    import concourse.mybir as mybir
    import concourse.tile as tile
    from concourse import bacc
    from contextlib import ExitStack

    f32 = mybir.dt.float32
    f16 = mybir.dt.float16
    AF = mybir.ActivationFunctionType
    ALU = mybir.AluOpType

    nc = bacc.Bacc("TRN2", target_bir_lowering=False)

    wb_cols = _wb_cols(t_steps)
    wb_d = nc.dram_tensor("wb", (128, wb_cols), f16, kind="ExternalInput")
    y_d = nc.dram_tensor("y", (BS, O), f32, kind="ExternalOutput")

    with tile.TileContext(nc) as tc, ExitStack() as ctx:
        const = ctx.enter_context(tc.tile_pool(name="const", bufs=1))
        cst = ctx.enter_context(tc.tile_pool(name="cst", bufs=3))
        work = ctx.enter_context(tc.tile_pool(name="work", bufs=3))
        hts = ctx.enter_context(tc.tile_pool(name="hts", bufs=3))
        psum = ctx.enter_context(tc.tile_pool(name="psum", bufs=3, space="PSUM"))

        wb = const.tile([128, wb_cols], f16)
        nc.sync.dma_start(wb[:], wb_d[:])

        def xt_ap(t):
            return wb[0:65, OFF_XT + BS * t : OFF_XT + BS * t + BS]

        def w0_ap(kc, m):
            return wb[:, OFF_W0 + 1024 * kc + 256 * m : OFF_W0 + 1024 * kc + 256 * m + 256]

        def w1_ap(kc, m):
            return wb[:, OFF_W1 + 1024 * kc + 256 * m : OFF_W1 + 1024 * kc + 256 * m + 256]

        def wx1_ap(kc, m):
            return wb[:, OFF_WX1 + 1024 * kc + 256 * m : OFF_WX1 + 1024 * kc + 256 * m + 256]

        def wx0_ap(m):
            return wb[0:65, OFF_WX0 + 256 * m : OFF_WX0 + 256 * m + 256]

        def b1_ap(m):
            return wb[0:1, OFF_B1 + 256 * m : OFF_B1 + 256 * m + 256]

        c0 = const.tile([128, 64], f16)
        c1 = const.tile([128, 64], f16)
        nc.vector.memset(c0[:], 0.0)
        nc.vector.memset(c1[:], 0.0)
        hT0 = hts.tile([128, 64], f16, tag="ht0")
        hT1 = hts.tile([128, 64], f16, tag="ht1")
        nc.vector.memset(hT0[:], 0.0)
        nc.vector.memset(hT1[:], 0.0)
        ones_t = const.tile([1, BS], f16)
        nc.vector.memset(ones_t[:], 1.0)
        ones_ap = ones_t[:]

        def elementwise(g, c_prev, tagsuf):
            # g cols: 0:64=i, 64:128=f, 128:192=o, 192:256=2*g_pre
            sg = work.tile([128, 256], f16, tag="sg" + tagsuf)
            nc.scalar.activation(sg[:], g[:], AF.Sigmoid)
            # m1 = (sig(2g) - 0.5) * sig(i)
            m1 = work.tile([128, 64], f16, tag="m1" + tagsuf)
            nc.vector.scalar_tensor_tensor(
                m1[:], sg[:, 192:256], 0.5, sg[:, 0:64], ALU.subtract, ALU.mult)
            # cf = sig(f) * C_prev
            cf = work.tile([128, 64], f16, tag="cf" + tagsuf)
            nc.vector.tensor_mul(cf[:], sg[:, 64:128], c_prev[:])
            # C = 4*m1 + cf
            c_new = cst.tile([128, 64], f16, tag="c" + tagsuf)
            nc.vector.scalar_tensor_tensor(
                c_new[:], m1[:], 4.0, cf[:], ALU.mult, ALU.add)
            # sc = sig(C) (= (tanh(c)+1)/2)
            sc = work.tile([128, 64], f16, tag="sc" + tagsuf)
            nc.scalar.activation(sc[:], c_new[:], AF.Sigmoid)
            # h' = (sc - 0.5) * sig(o)   (= h/2)
            h = work.tile([128, 64], f16, tag="h" + tagsuf)
            nc.vector.scalar_tensor_tensor(
                h[:], sc[:], 0.5, sg[:, 128:192], ALU.subtract, ALU.mult)
            hT = hts.tile([128, 64], f16, tag="ht" + tagsuf)
            nc.vector.transpose(hT[:], h[:])
            return hT, c_new

        def step0(t, hT0_prev, c_prev):
            g = psum.tile([128, 256], f32, tag="g0")
            for m in range(4):
                nc.tensor.matmul(
                    g[32 * m : 32 * m + 32, :], xt_ap(t), wx0_ap(m),
                    start=True, stop=False, tile_position=(0, 32 * m), skip_group_check=True,
                )
            for kc in range(2):
                for m in range(4):
                    nc.tensor.matmul(
                        g[32 * m : 32 * m + 32, :],
                        hT0_prev[:, 32 * kc : 32 * kc + 32], w0_ap(kc, m),
                        start=False, stop=(kc == 1), tile_position=(0, 32 * m), skip_group_check=True,
                    )
            return elementwise(g, c_prev, "0")

        def step1(hT0_t, hT1_prev, c_prev):
            g = psum.tile([128, 256], f32, tag="g1")
            for m in range(4):
                nc.tensor.matmul(
                    g[32 * m : 32 * m + 32, :], ones_ap, b1_ap(m),
                    start=True, stop=False, tile_position=(0, 32 * m), skip_group_check=True,
                )
            for src, w_ap in ((hT0_t, wx1_ap), (hT1_prev, w1_ap)):
                last_src = w_ap is w1_ap
                for kc in range(2):
                    for m in range(4):
                        nc.tensor.matmul(
                            g[32 * m : 32 * m + 32, :],
                            src[:, 32 * kc : 32 * kc + 32], w_ap(kc, m),
                            start=False,
                            stop=(last_src and kc == 1),
                            tile_position=(0, 32 * m), skip_group_check=True,
                        )
            return elementwise(g, c_prev, "1")

        hT0_hist = [hT0]
        for t in range(t_steps):
            hT0_new, c0 = step0(t, hT0_hist[-1], c0)
            hT0_hist.append(hT0_new)
            # layer 1 lags by one step so the two chains overlap
            if t >= 1:
                hT1, c1 = step1(hT0_hist[-2], hT1, c1)
            if len(hT0_hist) > 3:
                hT0_hist.pop(0)
        hT1, c1 = step1(hT0_hist[-1], hT1, c1)

        yp = psum.tile([BS, O], f32, tag="yh", bufs=1)
        nc.tensor.matmul(yp[:], ones_ap, wb[0:1, OFF_BF : OFF_BF + O], start=True, stop=False)
        nc.tensor.matmul(yp[:], hT1[:, 0:32], wb[:, OFF_WF : OFF_WF + O], start=False, stop=False)
        nc.tensor.matmul(yp[:], hT1[:, 32:64], wb[:, OFF_WF + O : OFF_WF + 2 * O], start=False, stop=True)
        y_sb = work.tile([BS, O], f32, tag="y")
        nc.vector.tensor_copy(y_sb[:], yp[:])
        nc.sync.dma_start(y_d[:], y_sb[:])

    return nc


def _scaled(W, b, hin_scale):
    """Apply the sigmoid-only folding scales to a weight [4H, K] and bias
    [4H] in ORIGINAL (i, f, g, o) gate order: g-gate rows x2 (sigmoid(2x)
    pre-scale) and the whole thing x hin_scale (h' = h/2 compensation)."""
    W = np.asarray(W, np.float64).copy()
    b = np.asarray(b, np.float64).copy() if b is not None else None
    W[2 * H : 3 * H] *= 2.0
    W *= hin_scale
    if b is not None:
        b[2 * H : 3 * H] *= 2.0
    return W, b


def _prep_inputs(x, Wih0, Whh0, bih0, bhh0, Wih1, Whh1, bih1, bhh1, W1, b1, W2, b2,
                 t_steps=T):
    x = np.asarray(x, dtype=np.float32)[:, :t_steps, :]
    wb = np.zeros((128, _wb_cols(t_steps)), np.float64)
    sWhh0, _ = _scaled(Whh0, None, 2.0)
    sWih0, sb0 = _scaled(Wih0, np.asarray(bih0, np.float64) + np.asarray(bhh0, np.float64), 1.0)
    sWhh1, _ = _scaled(Whh1, None, 2.0)
    sWih1, sb1 = _scaled(Wih1, np.asarray(bih1, np.float64) + np.asarray(bhh1, np.float64), 2.0)

    wb[:, OFF_W0 : OFF_W0 + 2048] = _perm_cols(
        sWhh0.T).reshape(2, 128, 1024).transpose(1, 0, 2).reshape(128, 2048)
    wb[:, OFF_W1 : OFF_W1 + 2048] = _perm_cols(
        sWhh1.T).reshape(2, 128, 1024).transpose(1, 0, 2).reshape(128, 2048)
    wb[:, OFF_WX1 : OFF_WX1 + 2048] = _perm_cols(
        sWih1.T).reshape(2, 128, 1024).transpose(1, 0, 2).reshape(128, 2048)
    wb[0:64, OFF_WX0 : OFF_WX0 + 1024] = _perm_cols(sWih0.T)
    wb[64, OFF_WX0 : OFF_WX0 + 1024] = _perm_cols(sb0[None, :])[0]
    wb[0, OFF_B1 : OFF_B1 + 1024] = _perm_cols(sb1[None, :])[0]
    # head folded: y = h2'*(2*W1.T@W2.T) + (b1@W2.T + b2)
    Wf = 2.0 * (np.asarray(W1, np.float64).T @ np.asarray(W2, np.float64).T)
    wb[:, OFF_WF : OFF_WF + 2 * O] = Wf.reshape(2, 128, O).transpose(1, 0, 2).reshape(128, 2 * O)
    wb[0, OFF_BF : OFF_BF + O] = (
        np.asarray(b1, np.float64) @ np.asarray(W2, np.float64).T + np.asarray(b2, np.float64))
    wb = wb.astype(np.float16)

    in_maps = []
    for c in range(NCORES):
        xc = x[c * BS : (c + 1) * BS]                       # [BS, t, I]
        xt = xc.transpose(2, 1, 0).reshape(I, t_steps * BS) # [I, t*BS]
        wbc = wb.copy()
        wbc[0:64, OFF_XT:] = xt.astype(np.float16)
        wbc[64, OFF_XT:] = 1.0
        in_maps.append(dict(wb=wbc))
    return in_maps


def run(t_steps=T, trace=False, **inputs):
    from concourse.bass_utils import run_bass_kernel_spmd

    key = t_steps
    if key not in _CACHED:
        nc_new = _build_bass(t_steps)
        # finalize BEFORE handing to the PJRT path: the bass_exec lowering
        # otherwise finalizes with the partition-id register preamble in a
        # state that miscompiles (walrus "Reg has not been allocated yet")
        nc_new.finalize()
        _CACHED[key] = nc_new
    nc = _CACHED[key]
    in_maps = _prep_inputs(**inputs, t_steps=t_steps)
    res = None
    for attempt in range(4):
        try:
            res = run_bass_kernel_spmd(nc, in_maps, core_ids=list(range(NCORES)),
                                       trace=trace)
            break
        except Exception as e:  # flaky parallel-birverifier race in neuronx-cc
            if attempt == 3:
                raise
            print(f"run attempt {attempt} failed ({type(e).__name__}); retrying")
    assert res is not None
    y = np.concatenate([r["y"] for r in res.results], axis=0)
    return y, res


def kernel(**inputs):
    y, _ = run(t_steps=T, trace=False, **inputs)
    return y


# revision 3
# speedup vs baseline: 1.0028x; 1.0028x over previous
"""Trainium2 Bass kernel for a 2-layer LSTM (B=256, T=512, I=64, H=256) + linear head.

Strategy (hardcoded, self-contained):
  - Data-parallel over batch across 8 NeuronCores (32 batch elems per core).
  - Per core, both LSTM layers run step-by-step in a feature-blocked layout:
      gate PSUM tile [128=(hblk4, b32), 256=(gate4, hh2, hl32)]
    produced by col-group-packed fp16 matmuls (tile_position=(0, 32*m)) that
    share the small transposed-state stationary hT [k, 32].
  - All matmul operands are fp16 (PSUM accumulation stays fp32): 4x faster
    streaming than fp32 on the PE.
  - The gate nonlinearity is a SINGLE sigmoid per gate tile: tanh is folded
    into sigmoid via tanh(x) = 2*sigmoid(2x) - 1, with the 2x pre-scale folded
    into the g-gate weight columns, the cell state kept as C = 2c, and the
    output scale folded into downstream weights via h' = h/2 (consumer
    weights x2).
  - Elementwise runs as fused scalar_tensor_tensor DVE ops in fp16 (4x DVE
    mode: 2-byte dtype, all-SBUF operands).
  - Input projection x@Wih.T and biases ride the same PSUM accumulation
    (augmented ones-row trick).
  - The two output linear layers are folded host-side into a single
    [256, 4] matmul + bias.
  - All weights ship as ONE packed fp16 DRAM blob -> one DMA.
"""

import numpy as np

B, T, I, H, O = 256, 512, 64, 256, 4
NCORES = 8
BS = B // NCORES  # 32

# reference gate order is (i, f, g, o); we reorder to (i, f, o, g) so that
# cols 0:64=i, 64:128=f, 128:192=o, 192:256=g per m-block.
GATE_PERM = [0, 1, 3, 2]

# weight blob column offsets (fp16 elements, [128, WB_COLS])
OFF_W0 = 0        # Whh0 perm  [128, 2*1024]
OFF_W1 = 2048     # Whh1 perm  [128, 2*1024]
OFF_WX1 = 4096    # Wih1 perm  [128, 2*1024]
OFF_WX0 = 6144    # Wih0 perm + bias row, rows 0:65, [65, 1024]
OFF_B1 = 7168     # bias1 row, row 0, [1, 1024]
OFF_WF = 8192     # folded head weight [128, 2*4]
OFF_BF = 8200     # folded head bias, row 0, [1, 4]
OFF_XT = 8224     # x transposed + ones row, rows 0:65, [65, t_steps*32]
def _wb_cols(t_steps):
    return OFF_XT + t_steps * BS

_CACHED = {}


def _perm_cols(Wt):
    """Permute gate columns of [K, 1024] (col j = gate_orig*256 + h) into
    col = m*256 + gate_new*64 + hh*32 + hl, where h = hh*128 + m*32 + hl."""
    K = Wt.shape[0]
    W = Wt.reshape(K, 4, 256)[:, GATE_PERM, :]      # [K, gate, h]
    W = W.reshape(K, 4, 2, 4, 32)                    # [K, gate, hh, m, hl]
    W = W.transpose(0, 3, 1, 2, 4)                   # [K, m, gate, hh, hl]
    return np.ascontiguousarray(W.reshape(K, 1024))


def _build_bass(t_steps=T):
    import concourse.mybir as mybir
    import concourse.tile as tile
    from concourse import bacc
    from contextlib import ExitStack

    f32 = mybir.dt.float32
    f16 = mybir.dt.float16
    bf16 = mybir.dt.bfloat16
    AF = mybir.ActivationFunctionType
    ALU = mybir.AluOpType

    nc = bacc.Bacc("TRN2", target_bir_lowering=False)

    wb_cols = _wb_cols(t_steps)
    wb_d = nc.dram_tensor("wb", (128, wb_cols), bf16, kind="ExternalInput")
    y_d = nc.dram_tensor("y", (BS, O), f32, kind="ExternalOutput")

    with tile.TileContext(nc) as tc, ExitStack() as ctx:
        const = ctx.enter_context(tc.tile_pool(name="const", bufs=1))
        cst = ctx.enter_context(tc.tile_pool(name="cst", bufs=3))
        work = ctx.enter_context(tc.tile_pool(name="work", bufs=3))
        hts = ctx.enter_context(tc.tile_pool(name="hts", bufs=3))
        psum = ctx.enter_context(tc.tile_pool(name="psum", bufs=3, space="PSUM"))

        wb = const.tile([128, wb_cols], bf16)
        nc.sync.dma_start(wb[:], wb_d[:])

        def xt_ap(t):
            return wb[0:65, OFF_XT + BS * t : OFF_XT + BS * t + BS]

        def w0_ap(kc, m):
            return wb[:, OFF_W0 + 1024 * kc + 256 * m : OFF_W0 + 1024 * kc + 256 * m + 256]

        def w1_ap(kc, m):
            return wb[:, OFF_W1 + 1024 * kc + 256 * m : OFF_W1 + 1024 * kc + 256 * m + 256]

        def wx1_ap(kc, m):
            return wb[:, OFF_WX1 + 1024 * kc + 256 * m : OFF_WX1 + 1024 * kc + 256 * m + 256]

        def wx0_ap(m):
            return wb[0:65, OFF_WX0 + 256 * m : OFF_WX0 + 256 * m + 256]

        def b1_ap(m):
            return wb[0:1, OFF_B1 + 256 * m : OFF_B1 + 256 * m + 256]

        c0 = const.tile([128, 64], f16)
        c1 = const.tile([128, 64], f16)
        nc.vector.memset(c0[:], 0.0)
        nc.vector.memset(c1[:], 0.0)
        hT0 = hts.tile([128, 64], bf16, tag="ht0")
        hT1 = hts.tile([128, 64], bf16, tag="ht1")
        nc.vector.memset(hT0[:], 0.0)
        nc.vector.memset(hT1[:], 0.0)
        ones_t = const.tile([1, BS], bf16)
        nc.vector.memset(ones_t[:], 1.0)
        ones_ap = ones_t[:]

        def elementwise(g, c_prev, tagsuf):
            # g cols: 0:64=i, 64:128=f, 128:192=o, 192:256=2*g_pre
            sg = work.tile([128, 256], f16, tag="sg" + tagsuf)
            nc.scalar.activation(sg[:], g[:], AF.Sigmoid)
            # m1 = (sig(2g) - 0.5) * sig(i)
            m1 = work.tile([128, 64], f16, tag="m1" + tagsuf)
            nc.vector.scalar_tensor_tensor(
                m1[:], sg[:, 192:256], 0.5, sg[:, 0:64], ALU.subtract, ALU.mult)
            # cf = sig(f) * C_prev
            cf = work.tile([128, 64], f16, tag="cf" + tagsuf)
            nc.vector.tensor_mul(cf[:], sg[:, 64:128], c_prev[:])
            # C = 4*m1 + cf
            c_new = cst.tile([128, 64], f16, tag="c" + tagsuf)
            nc.vector.scalar_tensor_tensor(
                c_new[:], m1[:], 4.0, cf[:], ALU.mult, ALU.add)
            # sc = sig(C) (= (tanh(c)+1)/2)
            sc = work.tile([128, 64], f16, tag="sc" + tagsuf)
            nc.scalar.activation(sc[:], c_new[:], AF.Sigmoid)
            # h' = (sc - 0.5) * sig(o)   (= h/2)
            h = work.tile([128, 64], bf16, tag="h" + tagsuf)
            nc.vector.scalar_tensor_tensor(
                h[:], sc[:], 0.5, sg[:, 128:192], ALU.subtract, ALU.mult)
            hT = hts.tile([128, 64], bf16, tag="ht" + tagsuf)
            nc.vector.transpose(hT[:], h[:])
            return hT, c_new

        def step0(t, hT0_prev, c_prev):
            g = psum.tile([128, 256], f32, tag="g0")
            for m in range(4):
                nc.tensor.matmul(
                    g[32 * m : 32 * m + 32, :], xt_ap(t), wx0_ap(m),
                    start=True, stop=False, tile_position=(0, 32 * m), skip_group_check=True,
                )
            for kc in range(2):
                for m in range(4):
                    nc.tensor.matmul(
                        g[32 * m : 32 * m + 32, :],
                        hT0_prev[:, 32 * kc : 32 * kc + 32], w0_ap(kc, m),
                        start=False, stop=(kc == 1), tile_position=(0, 32 * m), skip_group_check=True,
                    )
            return elementwise(g, c_prev, "0")

        def step1(hT0_t, hT1_prev, c_prev):
            g = psum.tile([128, 256], f32, tag="g1")
            for m in range(4):
                nc.tensor.matmul(
                    g[32 * m : 32 * m + 32, :], ones_ap, b1_ap(m),
                    start=True, stop=False, tile_position=(0, 32 * m), skip_group_check=True,
                )
            for src, w_ap in ((hT0_t, wx1_ap), (hT1_prev, w1_ap)):
                last_src = w_ap is w1_ap
                for kc in range(2):
                    for m in range(4):
                        nc.tensor.matmul(
                            g[32 * m : 32 * m + 32, :],
                            src[:, 32 * kc : 32 * kc + 32], w_ap(kc, m),
                            start=False,
                            stop=(last_src and kc == 1),
                            tile_position=(0, 32 * m), skip_group_check=True,
                        )
            return elementwise(g, c_prev, "1")

        hT0_hist = [hT0]
        for t in range(t_steps):
            hT0_new, c0 = step0(t, hT0_hist[-1], c0)
            hT0_hist.append(hT0_new)
            # layer 1 lags by one step so the two chains overlap
            if t >= 1:
                hT1, c1 = step1(hT0_hist[-2], hT1, c1)
            if len(hT0_hist) > 3:
                hT0_hist.pop(0)
        hT1, c1 = step1(hT0_hist[-1], hT1, c1)

        yp = psum.tile([BS, O], f32, tag="yh", bufs=1)
        nc.tensor.matmul(yp[:], ones_ap, wb[0:1, OFF_BF : OFF_BF + O], start=True, stop=False)
        nc.tensor.matmul(yp[:], hT1[:, 0:32], wb[:, OFF_WF : OFF_WF + O], start=False, stop=False)
        nc.tensor.matmul(yp[:], hT1[:, 32:64], wb[:, OFF_WF + O : OFF_WF + 2 * O], start=False, stop=True)
        y_sb = work.tile([BS, O], f32, tag="y")
        nc.vector.tensor_copy(y_sb[:], yp[:])
        nc.sync.dma_start(y_d[:], y_sb[:])

    return nc


def _scaled(W, b, hin_scale):
    """Apply the sigmoid-only folding scales to a weight [4H, K] and bias
    [4H] in ORIGINAL (i, f, g, o) gate order: g-gate rows x2 (sigmoid(2x)
    pre-scale) and the whole thing x hin_scale (h' = h/2 compensation)."""
    W = np.asarray(W, np.float64).copy()
    b = np.asarray(b, np.float64).copy() if b is not None else None
    W[2 * H : 3 * H] *= 2.0
    W *= hin_scale
    if b is not None:
        b[2 * H : 3 * H] *= 2.0
    return W, b


def _prep_inputs(x, Wih0, Whh0, bih0, bhh0, Wih1, Whh1, bih1, bhh1, W1, b1, W2, b2,
                 t_steps=T):
    x = np.asarray(x, dtype=np.float32)[:, :t_steps, :]
    wb = np.zeros((128, _wb_cols(t_steps)), np.float64)
    sWhh0, _ = _scaled(Whh0, None, 2.0)
    sWih0, sb0 = _scaled(Wih0, np.asarray(bih0, np.float64) + np.asarray(bhh0, np.float64), 1.0)
    sWhh1, _ = _scaled(Whh1, None, 2.0)
    sWih1, sb1 = _scaled(Wih1, np.asarray(bih1, np.float64) + np.asarray(bhh1, np.float64), 2.0)

    wb[:, OFF_W0 : OFF_W0 + 2048] = _perm_cols(
        sWhh0.T).reshape(2, 128, 1024).transpose(1, 0, 2).reshape(128, 2048)
    wb[:, OFF_W1 : OFF_W1 + 2048] = _perm_cols(
        sWhh1.T).reshape(2, 128, 1024).transpose(1, 0, 2).reshape(128, 2048)
    wb[:, OFF_WX1 : OFF_WX1 + 2048] = _perm_cols(
        sWih1.T).reshape(2, 128, 1024).transpose(1, 0, 2).reshape(128, 2048)
    wb[0:64, OFF_WX0 : OFF_WX0 + 1024] = _perm_cols(sWih0.T)
    wb[64, OFF_WX0 : OFF_WX0 + 1024] = _perm_cols(sb0[None, :])[0]
    wb[0, OFF_B1 : OFF_B1 + 1024] = _perm_cols(sb1[None, :])[0]
    # head folded: y = h2'*(2*W1.T@W2.T) + (b1@W2.T + b2)
    Wf = 2.0 * (np.asarray(W1, np.float64).T @ np.asarray(W2, np.float64).T)
    wb[:, OFF_WF : OFF_WF + 2 * O] = Wf.reshape(2, 128, O).transpose(1, 0, 2).reshape(128, 2 * O)
    wb[0, OFF_BF : OFF_BF + O] = (
        np.asarray(b1, np.float64) @ np.asarray(W2, np.float64).T + np.asarray(b2, np.float64))
    import ml_dtypes
    wb = wb.astype(ml_dtypes.bfloat16)

    in_maps = []
    for c in range(NCORES):
        xc = x[c * BS : (c + 1) * BS]                       # [BS, t, I]
        xt = xc.transpose(2, 1, 0).reshape(I, t_steps * BS) # [I, t*BS]
        wbc = wb.copy()
        wbc[0:64, OFF_XT:] = xt.astype(ml_dtypes.bfloat16)
        wbc[64, OFF_XT:] = 1.0
        in_maps.append(dict(wb=wbc))
    return in_maps


def run(t_steps=T, trace=False, **inputs):
    from concourse.bass_utils import run_bass_kernel_spmd

    key = t_steps
    if key not in _CACHED:
        nc_new = _build_bass(t_steps)
        # finalize BEFORE handing to the PJRT path: the bass_exec lowering
        # otherwise finalizes with the partition-id register preamble in a
        # state that miscompiles (walrus "Reg has not been allocated yet")
        nc_new.finalize()
        _CACHED[key] = nc_new
    nc = _CACHED[key]
    in_maps = _prep_inputs(**inputs, t_steps=t_steps)
    res = None
    for attempt in range(4):
        try:
            res = run_bass_kernel_spmd(nc, in_maps, core_ids=list(range(NCORES)),
                                       trace=trace)
            break
        except Exception as e:  # flaky parallel-birverifier race in neuronx-cc
            if attempt == 3:
                raise
            print(f"run attempt {attempt} failed ({type(e).__name__}); retrying")
    assert res is not None
    y = np.concatenate([r["y"] for r in res.results], axis=0)
    return y, res


def kernel(**inputs):
    y, _ = run(t_steps=T, trace=False, **inputs)
    return y


# revision 10
# speedup vs baseline: 1.0037x; 1.0009x over previous
"""Trainium2 Bass kernel for a 2-layer LSTM (B=256, T=512, I=64, H=256) + linear head.

Strategy (hardcoded, self-contained):
  - Data-parallel over batch across 8 NeuronCores (32 batch elems per core).
  - Per core, both LSTM layers run step-by-step in a feature-blocked layout:
      gate PSUM tile [128=(hblk4, b32), 256=(gate4, hh2, hl32)]
    produced by col-group-packed fp16 matmuls (tile_position=(0, 32*m)) that
    share the small transposed-state stationary hT [k, 32].
  - All matmul operands are fp16 (PSUM accumulation stays fp32): 4x faster
    streaming than fp32 on the PE.
  - The gate nonlinearity is a SINGLE sigmoid per gate tile: tanh is folded
    into sigmoid via tanh(x) = 2*sigmoid(2x) - 1, with the 2x pre-scale folded
    into the g-gate weight columns, the cell state kept as C = 2c, and the
    output scale folded into downstream weights via h' = h/2 (consumer
    weights x2).
  - Elementwise runs as fused scalar_tensor_tensor DVE ops in fp16 (4x DVE
    mode: 2-byte dtype, all-SBUF operands).
  - Input projection x@Wih.T and biases ride the same PSUM accumulation
    (augmented ones-row trick).
  - The two output linear layers are folded host-side into a single
    [256, 4] matmul + bias.
  - All weights ship as ONE packed fp16 DRAM blob -> one DMA.
"""

import numpy as np

B, T, I, H, O = 256, 512, 64, 256, 4
NCORES = 8
BS = B // NCORES  # 32

# reference gate order is (i, f, g, o); we reorder to (i, f, o, g) so that
# cols 0:64=i, 64:128=f, 128:192=o, 192:256=g per m-block.
GATE_PERM = [0, 1, 3, 2]

# weight blob column offsets (fp16 elements, [128, WB_COLS])
OFF_W0 = 0        # Whh0 perm  [128, 2*1024]
OFF_W1 = 2048     # Whh1 perm  [128, 2*1024]
OFF_WX1 = 4096    # Wih1 perm  [128, 2*1024]
OFF_WX0 = 6144    # Wih0 perm + bias row, rows 0:65, [65, 1024]
OFF_B1 = 7168     # bias1 row, row 0, [1, 1024]
OFF_WF = 8192     # folded head weight [128, 2*4]
OFF_BF = 8200     # folded head bias, row 0, [1, 4]
OFF_XT = 8224     # x transposed + ones row, rows 0:65, [65, t_steps*32]
def _wb_cols(t_steps):
    return OFF_XT + t_steps * BS

_CACHED = {}


def _perm_cols(Wt):
    """Permute gate columns of [K, 1024] (col j = gate_orig*256 + h) into
    col = m*256 + gate_new*64 + hh*32 + hl, where h = hh*128 + m*32 + hl."""
    K = Wt.shape[0]
    W = Wt.reshape(K, 4, 256)[:, GATE_PERM, :]      # [K, gate, h]
    W = W.reshape(K, 4, 2, 4, 32)                    # [K, gate, hh, m, hl]
    W = W.transpose(0, 3, 1, 2, 4)                   # [K, m, gate, hh, hl]
    return np.ascontiguousarray(W.reshape(K, 1024))


def _build_bass(t_steps=T):
    import concourse.mybir as mybir
    import concourse.tile as tile
    from concourse import bacc
    from contextlib import ExitStack

    f32 = mybir.dt.float32
    f16 = mybir.dt.float16
    bf16 = mybir.dt.bfloat16
    AF = mybir.ActivationFunctionType
    ALU = mybir.AluOpType

    nc = bacc.Bacc("TRN2", target_bir_lowering=False)

    wb_cols = _wb_cols(t_steps)
    wb_d = nc.dram_tensor("wb", (128, wb_cols), bf16, kind="ExternalInput")
    y_d = nc.dram_tensor("y", (BS, O), f32, kind="ExternalOutput")

    with tile.TileContext(nc) as tc, ExitStack() as ctx:
        const = ctx.enter_context(tc.tile_pool(name="const", bufs=1))
        cst = ctx.enter_context(tc.tile_pool(name="cst", bufs=3))
        work = ctx.enter_context(tc.tile_pool(name="work", bufs=4))
        hts = ctx.enter_context(tc.tile_pool(name="hts", bufs=8))
        psum = ctx.enter_context(tc.tile_pool(name="psum", bufs=3, space="PSUM"))

        wb = const.tile([128, wb_cols], bf16)
        nc.sync.dma_start(wb[:], wb_d[:])

        def xt_ap(t):
            return wb[0:65, OFF_XT + BS * t : OFF_XT + BS * t + BS]

        def w0_ap(kc, m):
            return wb[:, OFF_W0 + 1024 * kc + 256 * m : OFF_W0 + 1024 * kc + 256 * m + 256]

        def w1_ap(kc, m):
            return wb[:, OFF_W1 + 1024 * kc + 256 * m : OFF_W1 + 1024 * kc + 256 * m + 256]

        def wx1_ap(kc, m):
            return wb[:, OFF_WX1 + 1024 * kc + 256 * m : OFF_WX1 + 1024 * kc + 256 * m + 256]

        def wx0_ap(m):
            return wb[0:65, OFF_WX0 + 256 * m : OFF_WX0 + 256 * m + 256]

        def b1_ap(m):
            return wb[0:1, OFF_B1 + 256 * m : OFF_B1 + 256 * m + 256]

        c0 = const.tile([128, 64], f16)
        c1 = const.tile([128, 64], f16)
        nc.vector.memset(c0[:], 0.0)
        nc.vector.memset(c1[:], 0.0)
        hT0 = hts.tile([128, 64], bf16, tag="ht0")
        hT1 = hts.tile([128, 64], bf16, tag="ht1")
        nc.vector.memset(hT0[:], 0.0)
        nc.vector.memset(hT1[:], 0.0)
        ones_t = const.tile([1, BS], bf16)
        nc.vector.memset(ones_t[:], 1.0)
        ones_ap = ones_t[:]

        def elementwise(g, c_prev, tagsuf):
            # g cols: 0:64=i, 64:128=f, 128:192=o, 192:256=2*g_pre
            sg = work.tile([128, 256], f16, tag="sg" + tagsuf)
            nc.scalar.activation(sg[:], g[:], AF.Sigmoid)
            # State is C' = c/2: C' = sig(f)*C' + (sig(2g)-0.5)*sig(i), and
            # tanh(c) = tanh(2*C') via the ACT engine's free input scale.
            # m1 = (sig(2g) - 0.5) * sig(i)       [DVE]
            m1 = work.tile([128, 64], f16, tag="m1" + tagsuf)
            nc.vector.scalar_tensor_tensor(
                m1[:], sg[:, 192:256], 0.5, sg[:, 0:64], ALU.subtract, ALU.mult)
            # cf = sig(f) * C_prev                [GPSIMD, overlaps m1]
            cf = work.tile([128, 64], f16, tag="cf" + tagsuf)
            nc.gpsimd.tensor_mul(cf[:], sg[:, 64:128], c_prev[:])
            # C' = m1 + cf                        [DVE, plain add]
            c_new = cst.tile([128, 64], f16, tag="c" + tagsuf)
            nc.vector.tensor_add(c_new[:], m1[:], cf[:])
            # tc = tanh(2*C') = tanh(c)           [ACT, same table set]
            sc = work.tile([128, 64], f16, tag="sc" + tagsuf)
            nc.scalar.activation(sc[:], c_new[:], AF.Tanh, scale=2.0)
            # h = sig(o) * tanh(c)                [DVE, plain mult]
            h = work.tile([128, 64], bf16, tag="h" + tagsuf)
            nc.vector.tensor_mul(h[:], sc[:], sg[:, 128:192])
            hT = hts.tile([128, 64], bf16, tag="ht" + tagsuf)
            nc.vector.transpose(hT[:], h[:])
            return hT, c_new

        def step0(t, hT0_prev, c_prev):
            g = psum.tile([128, 256], f32, tag="g0")
            for m in range(4):
                nc.tensor.matmul(
                    g[32 * m : 32 * m + 32, :], xt_ap(t), wx0_ap(m),
                    start=True, stop=False, tile_position=(0, 32 * m), skip_group_check=True,
                )
            for kc in range(2):
                for m in range(4):
                    nc.tensor.matmul(
                        g[32 * m : 32 * m + 32, :],
                        hT0_prev[:, 32 * kc : 32 * kc + 32], w0_ap(kc, m),
                        start=False, stop=(kc == 1), tile_position=(0, 32 * m), skip_group_check=True,
                    )
            return elementwise(g, c_prev, "0")

        def step1(hT0_t, hT1_prev, c_prev):
            g = psum.tile([128, 256], f32, tag="g1")
            for m in range(4):
                nc.tensor.matmul(
                    g[32 * m : 32 * m + 32, :], ones_ap, b1_ap(m),
                    start=True, stop=False, tile_position=(0, 32 * m), skip_group_check=True,
                )
            for src, w_ap in ((hT0_t, wx1_ap), (hT1_prev, w1_ap)):
                last_src = w_ap is w1_ap
                for kc in range(2):
                    for m in range(4):
                        nc.tensor.matmul(
                            g[32 * m : 32 * m + 32, :],
                            src[:, 32 * kc : 32 * kc + 32], w_ap(kc, m),
                            start=False,
                            stop=(last_src and kc == 1),
                            tile_position=(0, 32 * m), skip_group_check=True,
                        )
            return elementwise(g, c_prev, "1")

        # Layer 1 lags layer 0 by LAG steps: with lag >= 2 the two serial
        # chains decouple (L1's inputs are always ready), so their engine
        # work interleaves instead of serializing.
        LAG = 4
        hT0_hist = [hT0]
        for t in range(t_steps):
            hT0_new, c0 = step0(t, hT0_hist[-1], c0)
            hT0_hist.append(hT0_new)
            if t >= LAG:
                hT1, c1 = step1(hT0_hist[-(LAG + 1)], hT1, c1)
            if len(hT0_hist) > LAG + 2:
                hT0_hist.pop(0)
        for k in range(LAG, 0, -1):
            hT1, c1 = step1(hT0_hist[-k], hT1, c1)

        yp = psum.tile([BS, O], f32, tag="yh", bufs=1)
        nc.tensor.matmul(yp[:], ones_ap, wb[0:1, OFF_BF : OFF_BF + O], start=True, stop=False)
        nc.tensor.matmul(yp[:], hT1[:, 0:32], wb[:, OFF_WF : OFF_WF + O], start=False, stop=False)
        nc.tensor.matmul(yp[:], hT1[:, 32:64], wb[:, OFF_WF + O : OFF_WF + 2 * O], start=False, stop=True)
        y_sb = work.tile([BS, O], f32, tag="y")
        nc.vector.tensor_copy(y_sb[:], yp[:])
        nc.sync.dma_start(y_d[:], y_sb[:])

    return nc


def _scaled(W, b, hin_scale):
    """Apply the sigmoid-only folding scales to a weight [4H, K] and bias
    [4H] in ORIGINAL (i, f, g, o) gate order: g-gate rows x2 (sigmoid(2x)
    pre-scale) and the whole thing x hin_scale (h' = h/2 compensation)."""
    W = np.asarray(W, np.float64).copy()
    b = np.asarray(b, np.float64).copy() if b is not None else None
    W[2 * H : 3 * H] *= 2.0
    W *= hin_scale
    if b is not None:
        b[2 * H : 3 * H] *= 2.0
    return W, b


def _prep_inputs(x, Wih0, Whh0, bih0, bhh0, Wih1, Whh1, bih1, bhh1, W1, b1, W2, b2,
                 t_steps=T):
    x = np.asarray(x, dtype=np.float32)[:, :t_steps, :]
    wb = np.zeros((128, _wb_cols(t_steps)), np.float64)
    sWhh0, _ = _scaled(Whh0, None, 1.0)
    sWih0, sb0 = _scaled(Wih0, np.asarray(bih0, np.float64) + np.asarray(bhh0, np.float64), 1.0)
    sWhh1, _ = _scaled(Whh1, None, 1.0)
    sWih1, sb1 = _scaled(Wih1, np.asarray(bih1, np.float64) + np.asarray(bhh1, np.float64), 1.0)

    wb[:, OFF_W0 : OFF_W0 + 2048] = _perm_cols(
        sWhh0.T).reshape(2, 128, 1024).transpose(1, 0, 2).reshape(128, 2048)
    wb[:, OFF_W1 : OFF_W1 + 2048] = _perm_cols(
        sWhh1.T).reshape(2, 128, 1024).transpose(1, 0, 2).reshape(128, 2048)
    wb[:, OFF_WX1 : OFF_WX1 + 2048] = _perm_cols(
        sWih1.T).reshape(2, 128, 1024).transpose(1, 0, 2).reshape(128, 2048)
    wb[0:64, OFF_WX0 : OFF_WX0 + 1024] = _perm_cols(sWih0.T)
    wb[64, OFF_WX0 : OFF_WX0 + 1024] = _perm_cols(sb0[None, :])[0]
    wb[0, OFF_B1 : OFF_B1 + 1024] = _perm_cols(sb1[None, :])[0]
    # head folded: y = h2*(W1.T@W2.T) + (b1@W2.T + b2)
    Wf = np.asarray(W1, np.float64).T @ np.asarray(W2, np.float64).T
    wb[:, OFF_WF : OFF_WF + 2 * O] = Wf.reshape(2, 128, O).transpose(1, 0, 2).reshape(128, 2 * O)
    wb[0, OFF_BF : OFF_BF + O] = (
        np.asarray(b1, np.float64) @ np.asarray(W2, np.float64).T + np.asarray(b2, np.float64))
    import ml_dtypes
    wb = wb.astype(ml_dtypes.bfloat16)

    in_maps = []
    for c in range(NCORES):
        xc = x[c * BS : (c + 1) * BS]                       # [BS, t, I]
        xt = xc.transpose(2, 1, 0).reshape(I, t_steps * BS) # [I, t*BS]
        wbc = wb.copy()
        wbc[0:64, OFF_XT:] = xt.astype(ml_dtypes.bfloat16)
        wbc[64, OFF_XT:] = 1.0
        in_maps.append(dict(wb=wbc))
    return in_maps


def run(t_steps=T, trace=False, **inputs):
    from concourse.bass_utils import run_bass_kernel_spmd

    key = t_steps
    if key not in _CACHED:
        nc_new = _build_bass(t_steps)
        # finalize BEFORE handing to the PJRT path: the bass_exec lowering
        # otherwise finalizes with the partition-id register preamble in a
        # state that miscompiles (walrus "Reg has not been allocated yet")
        nc_new.finalize()
        _CACHED[key] = nc_new
    nc = _CACHED[key]
    in_maps = _prep_inputs(**inputs, t_steps=t_steps)
    res = None
    for attempt in range(4):
        try:
            res = run_bass_kernel_spmd(nc, in_maps, core_ids=list(range(NCORES)),
                                       trace=trace)
            break
        except Exception as e:  # flaky parallel-birverifier race in neuronx-cc
            if attempt == 3:
                raise
            print(f"run attempt {attempt} failed ({type(e).__name__}); retrying")
    assert res is not None
    y = np.concatenate([r["y"] for r in res.results], axis=0)
    return y, res


def kernel(**inputs):
    y, _ = run(t_steps=T, trace=False, **inputs)
    return y


# revision 11
# speedup vs baseline: 1.0383x; 1.0345x over previous
"""Trainium2 Bass kernel for a 2-layer LSTM (B=256, T=512, I=64, H=256) + linear head.

Strategy (hardcoded, self-contained):
  - Data-parallel over batch across 8 NeuronCores (32 batch elems per core).
  - Per core, both LSTM layers run step-by-step in a feature-blocked layout:
      gate PSUM tile [128=(hblk4, b32), 256=(gate4, hh2, hl32)]
    produced by col-group-packed fp16 matmuls (tile_position=(0, 32*m)) that
    share the small transposed-state stationary hT [k, 32].
  - All matmul operands are fp16 (PSUM accumulation stays fp32): 4x faster
    streaming than fp32 on the PE.
  - The gate nonlinearity is a SINGLE sigmoid per gate tile: tanh is folded
    into sigmoid via tanh(x) = 2*sigmoid(2x) - 1, with the 2x pre-scale folded
    into the g-gate weight columns, the cell state kept as C = 2c, and the
    output scale folded into downstream weights via h' = h/2 (consumer
    weights x2).
  - Elementwise runs as fused scalar_tensor_tensor DVE ops in fp16 (4x DVE
    mode: 2-byte dtype, all-SBUF operands).
  - Input projection x@Wih.T and biases ride the same PSUM accumulation
    (augmented ones-row trick).
  - The two output linear layers are folded host-side into a single
    [256, 4] matmul + bias.
  - All weights ship as ONE packed fp16 DRAM blob -> one DMA.
"""

import numpy as np

B, T, I, H, O = 256, 512, 64, 256, 4
NCORES = 8
BS = B // NCORES  # 32

# reference gate order is (i, f, g, o); we reorder to (i, f, o, g) so that
# cols 0:64=i, 64:128=f, 128:192=o, 192:256=g per m-block.
GATE_PERM = [0, 1, 3, 2]

# weight blob column offsets (fp16 elements, [128, WB_COLS])
OFF_W0 = 0        # Whh0 perm  [128, 2*1024]
OFF_W1 = 2048     # Whh1 perm  [128, 2*1024]
OFF_WX1 = 4096    # Wih1 perm  [128, 2*1024]
OFF_WX0 = 6144    # Wih0 perm + bias row, rows 0:65, [65, 1024]
OFF_B1 = 7168     # bias1 row, row 0, [1, 1024]
OFF_WF = 8192     # folded head weight [128, 2*4]
OFF_BF = 8200     # folded head bias, row 0, [1, 4]
OFF_XT = 8224     # x transposed + ones row, rows 0:65, [65, t_steps*32]
def _wb_cols(t_steps):
    return OFF_XT + t_steps * BS

_CACHED = {}


def _perm_cols(Wt):
    """Permute gate columns of [K, 1024] (col j = gate_orig*256 + h) into
    col = m*256 + gate_new*64 + hh*32 + hl, where h = hh*128 + m*32 + hl."""
    K = Wt.shape[0]
    W = Wt.reshape(K, 4, 256)[:, GATE_PERM, :]      # [K, gate, h]
    W = W.reshape(K, 4, 2, 4, 32)                    # [K, gate, hh, m, hl]
    W = W.transpose(0, 3, 1, 2, 4)                   # [K, m, gate, hh, hl]
    return np.ascontiguousarray(W.reshape(K, 1024))


def _build_bass(t_steps=T):
    import concourse.mybir as mybir
    import concourse.tile as tile
    from concourse import bacc
    from contextlib import ExitStack

    f32 = mybir.dt.float32
    f16 = mybir.dt.float16
    bf16 = mybir.dt.bfloat16
    AF = mybir.ActivationFunctionType
    ALU = mybir.AluOpType

    nc = bacc.Bacc("TRN2", target_bir_lowering=False)

    wb_cols = _wb_cols(t_steps)
    wb_d = nc.dram_tensor("wb", (128, wb_cols), bf16, kind="ExternalInput")
    y_d = nc.dram_tensor("y", (BS, O), f32, kind="ExternalOutput")

    with tile.TileContext(nc) as tc, ExitStack() as ctx:
        const = ctx.enter_context(tc.tile_pool(name="const", bufs=1))
        cst = ctx.enter_context(tc.tile_pool(name="cst", bufs=3))
        work = ctx.enter_context(tc.tile_pool(name="work", bufs=4))
        hts = ctx.enter_context(tc.tile_pool(name="hts", bufs=8))
        psum = ctx.enter_context(tc.tile_pool(name="psum", bufs=3, space="PSUM"))

        wb = const.tile([128, wb_cols], bf16)
        nc.sync.dma_start(wb[:], wb_d[:])

        def xt_ap(t):
            return wb[0:65, OFF_XT + BS * t : OFF_XT + BS * t + BS]

        def w0_ap(kc, m):
            return wb[:, OFF_W0 + 1024 * kc + 256 * m : OFF_W0 + 1024 * kc + 256 * m + 256]

        def w1_ap(kc, m):
            return wb[:, OFF_W1 + 1024 * kc + 256 * m : OFF_W1 + 1024 * kc + 256 * m + 256]

        def wx1_ap(kc, m):
            return wb[:, OFF_WX1 + 1024 * kc + 256 * m : OFF_WX1 + 1024 * kc + 256 * m + 256]

        def wx0_ap(m):
            return wb[0:65, OFF_WX0 + 256 * m : OFF_WX0 + 256 * m + 256]

        def b1_ap(m):
            return wb[0:1, OFF_B1 + 256 * m : OFF_B1 + 256 * m + 256]

        c0 = const.tile([128, 64], f16)
        c1 = const.tile([128, 64], f16)
        nc.vector.memset(c0[:], 0.0)
        nc.vector.memset(c1[:], 0.0)
        hT0 = hts.tile([128, 64], bf16, tag="ht0")
        hT1 = hts.tile([128, 64], bf16, tag="ht1")
        nc.vector.memset(hT0[:], 0.0)
        nc.vector.memset(hT1[:], 0.0)
        ones_t = const.tile([1, BS], bf16)
        nc.vector.memset(ones_t[:], 1.0)
        ones_ap = ones_t[:]

        def elementwise(g, c_prev, tagsuf):
            # g cols: 0:64=i, 64:128=f, 128:192=o, 192:256=2*g_pre
            sg = work.tile([128, 256], f16, tag="sg" + tagsuf)
            nc.scalar.activation(sg[:], g[:], AF.Sigmoid)
            # State is C' = c/2: C' = sig(f)*C' + (sig(2g)-0.5)*sig(i), and
            # tanh(c) = tanh(2*C') via the ACT engine's free input scale.
            # m1 = (sig(2g) - 0.5) * sig(i)       [DVE]
            m1 = work.tile([128, 64], f16, tag="m1" + tagsuf)
            nc.vector.scalar_tensor_tensor(
                m1[:], sg[:, 192:256], 0.5, sg[:, 0:64], ALU.subtract, ALU.mult)
            # cf = sig(f) * C_prev                [DVE]
            cf = work.tile([128, 64], f16, tag="cf" + tagsuf)
            nc.vector.tensor_mul(cf[:], sg[:, 64:128], c_prev[:])
            # C' = m1 + cf                        [DVE, plain add]
            c_new = cst.tile([128, 64], f16, tag="c" + tagsuf)
            nc.vector.tensor_add(c_new[:], m1[:], cf[:])
            # tc = tanh(2*C') = tanh(c)           [ACT, same table set]
            sc = work.tile([128, 64], f16, tag="sc" + tagsuf)
            nc.scalar.activation(sc[:], c_new[:], AF.Tanh, scale=2.0)
            # h = sig(o) * tanh(c)                [DVE, plain mult]
            h = work.tile([128, 64], bf16, tag="h" + tagsuf)
            nc.vector.tensor_mul(h[:], sc[:], sg[:, 128:192])
            hT = hts.tile([128, 64], bf16, tag="ht" + tagsuf)
            nc.vector.transpose(hT[:], h[:])
            return hT, c_new

        def step0(t, hT0_prev, c_prev):
            g = psum.tile([128, 256], f32, tag="g0")
            for m in range(4):
                nc.tensor.matmul(
                    g[32 * m : 32 * m + 32, :], xt_ap(t), wx0_ap(m),
                    start=True, stop=False, tile_position=(0, 32 * m), skip_group_check=True,
                )
            for kc in range(2):
                for m in range(4):
                    nc.tensor.matmul(
                        g[32 * m : 32 * m + 32, :],
                        hT0_prev[:, 32 * kc : 32 * kc + 32], w0_ap(kc, m),
                        start=False, stop=(kc == 1), tile_position=(0, 32 * m), skip_group_check=True,
                    )
            return elementwise(g, c_prev, "0")

        def step1(hT0_t, hT1_prev, c_prev):
            g = psum.tile([128, 256], f32, tag="g1")
            for m in range(4):
                nc.tensor.matmul(
                    g[32 * m : 32 * m + 32, :], ones_ap, b1_ap(m),
                    start=True, stop=False, tile_position=(0, 32 * m), skip_group_check=True,
                )
            for src, w_ap in ((hT0_t, wx1_ap), (hT1_prev, w1_ap)):
                last_src = w_ap is w1_ap
                for kc in range(2):
                    for m in range(4):
                        nc.tensor.matmul(
                            g[32 * m : 32 * m + 32, :],
                            src[:, 32 * kc : 32 * kc + 32], w_ap(kc, m),
                            start=False,
                            stop=(last_src and kc == 1),
                            tile_position=(0, 32 * m), skip_group_check=True,
                        )
            return elementwise(g, c_prev, "1")

        # Layer 1 lags layer 0 by LAG steps: with lag >= 2 the two serial
        # chains decouple (L1's inputs are always ready), so their engine
        # work interleaves instead of serializing.
        LAG = 4
        hT0_hist = [hT0]
        for t in range(t_steps):
            hT0_new, c0 = step0(t, hT0_hist[-1], c0)
            hT0_hist.append(hT0_new)
            if t >= LAG:
                hT1, c1 = step1(hT0_hist[-(LAG + 1)], hT1, c1)
            if len(hT0_hist) > LAG + 2:
                hT0_hist.pop(0)
        for k in range(LAG, 0, -1):
            hT1, c1 = step1(hT0_hist[-k], hT1, c1)

        yp = psum.tile([BS, O], f32, tag="yh", bufs=1)
        nc.tensor.matmul(yp[:], ones_ap, wb[0:1, OFF_BF : OFF_BF + O], start=True, stop=False)
        nc.tensor.matmul(yp[:], hT1[:, 0:32], wb[:, OFF_WF : OFF_WF + O], start=False, stop=False)
        nc.tensor.matmul(yp[:], hT1[:, 32:64], wb[:, OFF_WF + O : OFF_WF + 2 * O], start=False, stop=True)
        y_sb = work.tile([BS, O], f32, tag="y")
        nc.vector.tensor_copy(y_sb[:], yp[:])
        nc.sync.dma_start(y_d[:], y_sb[:])

    return nc


def _scaled(W, b, hin_scale):
    """Apply the sigmoid-only folding scales to a weight [4H, K] and bias
    [4H] in ORIGINAL (i, f, g, o) gate order: g-gate rows x2 (sigmoid(2x)
    pre-scale) and the whole thing x hin_scale (h' = h/2 compensation)."""
    W = np.asarray(W, np.float64).copy()
    b = np.asarray(b, np.float64).copy() if b is not None else None
    W[2 * H : 3 * H] *= 2.0
    W *= hin_scale
    if b is not None:
        b[2 * H : 3 * H] *= 2.0
    return W, b


def _prep_inputs(x, Wih0, Whh0, bih0, bhh0, Wih1, Whh1, bih1, bhh1, W1, b1, W2, b2,
                 t_steps=T):
    x = np.asarray(x, dtype=np.float32)[:, :t_steps, :]
    wb = np.zeros((128, _wb_cols(t_steps)), np.float64)
    sWhh0, _ = _scaled(Whh0, None, 1.0)
    sWih0, sb0 = _scaled(Wih0, np.asarray(bih0, np.float64) + np.asarray(bhh0, np.float64), 1.0)
    sWhh1, _ = _scaled(Whh1, None, 1.0)
    sWih1, sb1 = _scaled(Wih1, np.asarray(bih1, np.float64) + np.asarray(bhh1, np.float64), 1.0)

    wb[:, OFF_W0 : OFF_W0 + 2048] = _perm_cols(
        sWhh0.T).reshape(2, 128, 1024).transpose(1, 0, 2).reshape(128, 2048)
    wb[:, OFF_W1 : OFF_W1 + 2048] = _perm_cols(
        sWhh1.T).reshape(2, 128, 1024).transpose(1, 0, 2).reshape(128, 2048)
    wb[:, OFF_WX1 : OFF_WX1 + 2048] = _perm_cols(
        sWih1.T).reshape(2, 128, 1024).transpose(1, 0, 2).reshape(128, 2048)
    wb[0:64, OFF_WX0 : OFF_WX0 + 1024] = _perm_cols(sWih0.T)
    wb[64, OFF_WX0 : OFF_WX0 + 1024] = _perm_cols(sb0[None, :])[0]
    wb[0, OFF_B1 : OFF_B1 + 1024] = _perm_cols(sb1[None, :])[0]
    # head folded: y = h2*(W1.T@W2.T) + (b1@W2.T + b2)
    Wf = np.asarray(W1, np.float64).T @ np.asarray(W2, np.float64).T
    wb[:, OFF_WF : OFF_WF + 2 * O] = Wf.reshape(2, 128, O).transpose(1, 0, 2).reshape(128, 2 * O)
    wb[0, OFF_BF : OFF_BF + O] = (
        np.asarray(b1, np.float64) @ np.asarray(W2, np.float64).T + np.asarray(b2, np.float64))
    import ml_dtypes
    wb = wb.astype(ml_dtypes.bfloat16)

    in_maps = []
    for c in range(NCORES):
        xc = x[c * BS : (c + 1) * BS]                       # [BS, t, I]
        xt = xc.transpose(2, 1, 0).reshape(I, t_steps * BS) # [I, t*BS]
        wbc = wb.copy()
        wbc[0:64, OFF_XT:] = xt.astype(ml_dtypes.bfloat16)
        wbc[64, OFF_XT:] = 1.0
        in_maps.append(dict(wb=wbc))
    return in_maps


def run(t_steps=T, trace=False, **inputs):
    from concourse.bass_utils import run_bass_kernel_spmd

    key = t_steps
    if key not in _CACHED:
        nc_new = _build_bass(t_steps)
        # finalize BEFORE handing to the PJRT path: the bass_exec lowering
        # otherwise finalizes with the partition-id register preamble in a
        # state that miscompiles (walrus "Reg has not been allocated yet")
        nc_new.finalize()
        _CACHED[key] = nc_new
    nc = _CACHED[key]
    in_maps = _prep_inputs(**inputs, t_steps=t_steps)
    res = None
    for attempt in range(4):
        try:
            res = run_bass_kernel_spmd(nc, in_maps, core_ids=list(range(NCORES)),
                                       trace=trace)
            break
        except Exception as e:  # flaky parallel-birverifier race in neuronx-cc
            if attempt == 3:
                raise
            print(f"run attempt {attempt} failed ({type(e).__name__}); retrying")
    assert res is not None
    y = np.concatenate([r["y"] for r in res.results], axis=0)
    return y, res


def kernel(**inputs):
    y, _ = run(t_steps=T, trace=False, **inputs)
    return y
